# revision 19
# baseline (speedup 1.0000x reference)
"""NeuralCDE RK4 solver as a Bass/Tile kernel on 8 Trainium2 cores.

Data-parallel over batch: B=1024 -> 128 rows per core (one partition tile).
The 127-step RK4 scan is fully unrolled; per stage:
    mm1 (PE)  : h_psum[128m,128b] = W1z.T @ zT_stage
    relu (ACT): hS = relu(h_psum + bias1(t))     (time channel folded in bias)
    mm2 (PE)  : f_psum[128b,512]  = ones.T@b2 + hS.T @ W2   (accumulated)
    tanh (ACT): fS = tanh(f_psum)
    mul  (DVE): u = fS * g(step,stage)           (g broadcast along h via AP)
    red  (DVE): k_nat[128b,64] = sum_c u
    T    (PE) : k^T accumulated into acc_psum    (RK4 weights pre-folded in g)
    stt  (DVE): z_stage_next = k^T * alpha + zT
State z^T lives in one big SBUF buffer [64, 128*128] (slot per grid point);
slots stream out to DRAM as they finish.
"""

import numpy as np
import ml_dtypes

import concourse.bacc as bacc
import concourse.bass as bass
import concourse.mybir as mybir
from concourse.tile import TileContext
from concourse.bass_utils import run_bass_kernel_spmd

F32 = mybir.dt.float32
F32R = mybir.dt.float32r
BF16 = mybir.dt.bfloat16
FP16 = mybir.dt.float16
B = 1024
L = 128
C_IN = 8
HID = 64
MLP_H = 128
INIT_H = 20
NSTEP = L - 1  # 127
NCORES = 8
BL = B // NCORES  # 128 batch rows per core

_CACHE: dict = {}


def _flags():
    import os
    return (
        os.environ.get("K_T_F32R", "0") == "1",
        os.environ.get("K_MM2_F32R", "1") == "1",
        os.environ.get("K_MUL_BF16", "1") == "1",
        os.environ.get("K_MM1_F32R", "0") == "1",
        os.environ.get("K_WARM", "0") == "1",
        os.environ.get("K_FP16_PATH", "1") == "1",
    )


def _build(nstep: int, with_b2: bool):
    import time as _time

    t_f32r, mm2_f32r, mul_bf16, mm1_f32r, warm, fp16_path = _flags()
    TD = F32R if t_f32r else F32
    SD = F32R if mm1_f32r else F32
    MD = F32R if mm2_f32r else F32
    UD = BF16 if mul_bf16 else F32
    if fp16_path:
        MD = FP16
        UD = FP16
    t0 = _time.time()
    nc = bacc.Bacc()
    g_in = nc.dram_tensor("g", [BL, nstep * 3 * C_IN], UD, kind="ExternalInput")
    b1_in = nc.dram_tensor("bias1", [MLP_H, nstep * 3], F32, kind="ExternalInput")
    w1z_in = nc.dram_tensor("w1z", [HID, MLP_H], SD, kind="ExternalInput")
    w2_in = nc.dram_tensor("w2", [MLP_H, HID * C_IN], MD, kind="ExternalInput")
    b2_in = nc.dram_tensor("b2r", [1, HID * C_IN], MD, kind="ExternalInput")
    ones_in = nc.dram_tensor("onesr", [1, BL], MD, kind="ExternalInput")
    id_in = nc.dram_tensor("ident", [BL, BL], TD, kind="ExternalInput")
    z0t_in = nc.dram_tensor("z0t", [HID, BL], SD, kind="ExternalInput")
    zs_out = nc.dram_tensor(
        "zs", [HID, (nstep + 1) * BL], F32, kind="ExternalOutput"
    )

    NF = HID * C_IN  # 512
    with TileContext(nc) as tc:
        with (
            tc.tile_pool(name="const", bufs=1) as cp,
            tc.tile_pool(name="zst", bufs=1) as zp,
            tc.tile_pool(name="hs", bufs=3) as hp,
            tc.tile_pool(name="fs", bufs=2) as fp,
            tc.tile_pool(name="us", bufs=2) as up,
            tc.tile_pool(name="ks", bufs=3) as kp,
            tc.tile_pool(name="zc", bufs=3) as zcp,
            tc.tile_pool(name="ph", bufs=2, space="PSUM") as ph,
            tc.tile_pool(name="pf", bufs=2, space="PSUM") as pf,
            tc.tile_pool(name="pacc", bufs=2, space="PSUM") as pacc,
            tc.tile_pool(name="pks", bufs=1, space="PSUM") as pks,
            tc.tile_pool(name="pfill", bufs=1, space="PSUM") as pfill,
        ):
            gS = cp.tile([BL, nstep * 3 * C_IN], UD)
            b1S = cp.tile([MLP_H, nstep * 3], F32)
            w1zS = cp.tile([HID, MLP_H], SD)
            w2S = cp.tile([MLP_H, NF], MD)
            b2S = cp.tile([1, NF], MD)
            onesS = cp.tile([1, BL], MD)
            idS = cp.tile([BL, BL], TD)
            zall = zp.tile([HID, (nstep + 1) * BL], SD)
            if warm:
                wt = cp.tile([BL, BL], BF16, name="wt")
                nc.vector.memset(wt[:], 0.0)

            nc.sync.dma_start(out=gS[:], in_=g_in[:])
            nc.sync.dma_start(out=b1S[:], in_=b1_in[:])
            nc.sync.dma_start(out=w1zS[:], in_=w1z_in[:])
            nc.sync.dma_start(out=w2S[:], in_=w2_in[:])
            nc.sync.dma_start(out=b2S[:], in_=b2_in[:])
            nc.sync.dma_start(out=onesS[:], in_=ones_in[:])
            nc.sync.dma_start(out=idS[:], in_=id_in[:])
            nc.sync.dma_start(out=zall[:, 0:BL], in_=z0t_in[:])
            nc.sync.dma_start(out=zs_out[:, 0:BL], in_=z0t_in[:].bitcast(F32))

            if warm:
                wp = pfill.tile([BL, BL], F32, tag="fl", name="wp")
                for _ in range(48):
                    nc.tensor.matmul(
                        wp[:], lhsT=wt[:], rhs=wt[:], start=True, stop=True
                    )
            CLS = (0, 1, 1, 2)
            ALPHA = (0.5, 0.25, 0.5, 1.0 / 6.0)
            for step in range(nstep):
                zT = zall[:, step * BL : (step + 1) * BL]
                cur = zT
                accP = None
                for s in range(4):
                    col = step * 3 + CLS[s]
                    h_ps = ph.tile([MLP_H, BL], F32, tag="hps")
                    nc.tensor.matmul(
                        h_ps[:],
                        lhsT=w1zS[:],
                        rhs=cur,
                        start=True,
                        stop=True,
                    )
                    hS = hp.tile([MLP_H, BL], MD, tag="hs")
                    nc.vector.tensor_scalar(
                        hS[:],
                        h_ps[:],
                        b1S[:, col : col + 1],
                        0.0,
                        op0=mybir.AluOpType.add,
                        op1=mybir.AluOpType.max,
                    )
                    f_ps = pf.tile([BL, NF], F32, tag="fps")
                    if with_b2:
                        nc.tensor.matmul(
                            f_ps[:],
                            lhsT=onesS[:],
                            rhs=b2S[:],
                            start=True,
                            stop=False,
                        )
                    nc.tensor.matmul(
                        f_ps[:],
                        lhsT=hS[:],
                        rhs=w2S[:],
                        start=not with_b2,
                        stop=True,
                    )
                    fS = fp.tile([BL, NF], UD, tag="fs")
                    nc.scalar.activation(
                        fS[:], f_ps[:], mybir.ActivationFunctionType.Tanh
                    )
                    if warm:
                        fl1 = pfill.tile([BL, BL], F32, tag="fl", name="fl1")
                        nc.tensor.matmul(
                            fl1[:],
                            lhsT=fS[:, 0:BL],
                            rhs=fS[:, 0:BL],
                            start=True,
                            stop=True,
                        )
                    u = up.tile([BL, NF], UD, tag="u")
                    f3 = fS[:].rearrange("p (h c) -> p h c", c=C_IN)
                    u3 = u[:].rearrange("p (h c) -> p h c", c=C_IN)
                    gv = (
                        gS[:, col * C_IN : (col + 1) * C_IN]
                        .unsqueeze(1)
                        .broadcast_to((BL, HID, C_IN))
                    )
                    nc.vector.tensor_tensor(
                        out=u3, in0=f3, in1=gv, op=mybir.AluOpType.mult
                    )
                    if warm:
                        fl2 = pfill.tile([BL, BL], F32, tag="fl", name="fl2")
                        nc.tensor.matmul(
                            fl2[:],
                            lhsT=u[:, 0:BL],
                            rhs=u[:, 0:BL],
                            start=True,
                            stop=True,
                        )
                    kn = kp.tile([BL, HID], TD, tag="kn")
                    with nc.allow_low_precision("k quantized to bf16; error ~0.4% of k, negligible after RK4 averaging"):
                        nc.vector.tensor_reduce(
                            kn[:], u3, axis=mybir.AxisListType.X, op=mybir.AluOpType.add
                        )
                    if s == 0:
                        accP = pacc.tile([HID, BL], TD, tag="acc")
                        nc.tensor.matmul(
                            accP[:],
                            lhsT=kn[:],
                            rhs=idS[:],
                            is_transpose=True,
                            start=True,
                            stop=True,
                        )
                        src = accP
                    elif s in (1, 2):
                        ksP = pks.tile([HID, BL], TD, tag="ks")
                        nc.tensor.matmul(
                            ksP[:],
                            lhsT=kn[:],
                            rhs=idS[:],
                            is_transpose=True,
                            start=True,
                            stop=True,
                        )
                        nc.tensor.matmul(
                            accP[:],
                            lhsT=kn[:],
                            rhs=idS[:],
                            is_transpose=True,
                            start=False,
                            stop=True,
                            skip_group_check=True,
                        )
                        src = ksP
                    else:
                        nc.tensor.matmul(
                            accP[:],
                            lhsT=kn[:],
                            rhs=idS[:],
                            is_transpose=True,
                            start=False,
                            stop=True,
                            skip_group_check=True,
                        )
                        src = accP
                    if s < 3:
                        out_ap = zcp.tile([HID, BL], SD, tag="zc", name="zc")[:]
                    else:
                        out_ap = zall[:, (step + 1) * BL : (step + 2) * BL]
                    nc.vector.scalar_tensor_tensor(
                        out=out_ap,
                        in0=src[:],
                        scalar=ALPHA[s],
                        in1=zT,
                        op0=mybir.AluOpType.mult,
                        op1=mybir.AluOpType.add,
                    )
                    if s < 3:
                        cur = out_ap
                nc.sync.dma_start(
                    out=zs_out[:, (step + 1) * BL : (step + 2) * BL],
                    in_=zall[:, (step + 1) * BL : (step + 2) * BL].bitcast(F32),
                )
    import sys

    print(f"[kernel] tile trace+schedule: {_time.time()-t0:.1f}s", file=sys.stderr)
    t1 = _time.time()
    nc.finalize()
    print(f"[kernel] finalize: {_time.time()-t1:.1f}s", file=sys.stderr)
    return nc


def _get_nc(nstep: int, with_b2: bool):
    key = (nstep, with_b2) + _flags()
    if key not in _CACHE:
        _CACHE[key] = _build(nstep, with_b2)
    return _CACHE[key]


def _host_prep(coeffs, Wi1, bi1, Wi2, bi2, W1, b1, W2, b2, nstep: int):
    coeffs = np.asarray(coeffs, dtype=np.float32)
    a = coeffs[:, :, 0:8]
    b = coeffs[:, :, 8:16]
    c = coeffs[:, :, 16:24]
    d = coeffs[:, :, 24:32]

    X0 = a[:, 0]
    z0 = np.tanh(
        np.maximum(X0 @ Wi1 + bi1, 0.0).astype(np.float32) @ Wi2 + bi2
    ).astype(np.float32)

    g = np.empty((B, nstep, 3, C_IN), dtype=np.float32)
    g[:, :, 0] = b[:, :nstep]
    g[:, :, 1] = 2.0 * b[:, :nstep] + 2.0 * c[:, :nstep] + 1.5 * d[:, :nstep]
    # stage-4 derivative: dXdt at t=i+1
    last = NSTEP - 1  # 126 in full problem
    for i in range(nstep):
        if i < last:
            g[:, i, 2] = b[:, i + 1]
        else:
            g[:, i, 2] = b[:, i] + 2.0 * c[:, i] + 3.0 * d[:, i]

    tcols = np.empty((nstep, 3), dtype=np.float32)
    tcols[:, 0] = np.arange(nstep, dtype=np.float32)
    tcols[:, 1] = tcols[:, 0] + 0.5
    tcols[:, 2] = tcols[:, 0] + 1.0
    # bias1[m, step*3+cls] = b1[m] + t * W1[0, m]
    bias1 = (
        b1[None, None, :] + tcols[:, :, None] * W1[0][None, None, :]
    ).astype(np.float32)
    bias1 = bias1.reshape(nstep * 3, MLP_H).T.copy()  # [128, nstep*3]

    wdt = np.float16 if _flags()[5] else np.float32
    shared = {
        "bias1": bias1,
        "w1z": np.ascontiguousarray(W1[1:], dtype=np.float32),
        "w2": np.ascontiguousarray(W2, dtype=wdt),
        "b2r": np.ascontiguousarray(b2[None, :], dtype=wdt),
        "onesr": np.ones((1, BL), dtype=wdt),
        "ident": np.eye(BL, dtype=np.float32),
    }
    in_maps = []
    for core in range(NCORES):
        sl = slice(core * BL, (core + 1) * BL)
        m = dict(shared)
        f = _flags()
        gdt = np.float16 if f[5] else (ml_dtypes.bfloat16 if f[2] else np.float32)
        m["g"] = np.ascontiguousarray(
            g[sl].reshape(BL, nstep * 3 * C_IN).astype(gdt)
        )
        m["z0t"] = np.ascontiguousarray(z0[sl].T)
        in_maps.append(m)
    return in_maps, z0


def kernel(coeffs, Wi1, bi1, Wi2, bi2, W1, b1, W2, b2, _nstep: int = NSTEP,
           _trace: bool = False):
    import time as _time
    import sys

    nstep = _nstep
    with_b2 = bool(np.any(np.asarray(b2)))
    nc = _get_nc(nstep, with_b2)
    in_maps, _ = _host_prep(
        coeffs, Wi1, bi1, Wi2, bi2, W1, b1, W2, b2, nstep
    )
    t0 = _time.time()
    res = run_bass_kernel_spmd(nc, in_maps, list(range(NCORES)), trace=_trace)
    print(f"[kernel] spmd run (compile+exec): {_time.time()-t0:.1f}s", file=sys.stderr)
    out = np.empty((B, nstep + 1, HID), dtype=np.float32)
    for core in range(NCORES):
        zs = res.results[core]["zs"].reshape(HID, nstep + 1, BL)
        out[core * BL : (core + 1) * BL] = zs.transpose(2, 1, 0)
    if _trace:
        kernel.last_results = res
    return out
